# revision 1
# baseline (speedup 1.0000x reference)
"""MetaPathConnector kernel for Trainium2 (8 NeuronCores, Bass/Tile), v4.

Row-shards N=16384 nodes across 8 cores (2048 rows each). Each core receives a
rotated copy of feat^T (bf16) so its own rows occupy columns [0, 2048) -- the
device program is identical (static) on every core.

Structure:
  prep:  proj = W @ featT in bf16 -> projT fp32 in SBUF; PE transposes to
         natural layout; row norms; nrmT bf16 (sims operand); fp32 proj rows
         to DRAM (gather source).
  main per 128-row tile:
         8 blocks x [128,2048] PSUM: bf16 matmuls (+ -8*I accumulated onto
         the diagonal); DVE MAX8 -> top-8 values, DVE FIND_INDEX8 -> local
         columns (both read PSUM; no SBUF staging).
         candidates packed as (value_bits & -2048) | local_idx  (int32) so
         the 64-wide refinement (MAX8 / MATCH_REPLACE8 / MAX8 + FIND_INDEX8)
         yields exact top-10 values AND indices without one-hot recovery:
         gidx = ((slot & ~7) << 8) | (packed & 0x7FF).
  gather/apply: software-pipelined 2 tiles behind topk so dma_gather (Q7)
         and the DVE apply overlap the scans instead of tailing the kernel.
"""

from contextlib import ExitStack

import numpy as np
import ml_dtypes

import concourse.bass as bass
import concourse.mybir as mybir
import concourse.tile as tile
from concourse import bacc
from concourse.bass_utils import run_bass_kernel_spmd
from concourse.masks import make_identity

FP32 = mybir.dt.float32
BF16 = mybir.dt.bfloat16
I32 = mybir.dt.int32
I16 = mybir.dt.int16
U16 = mybir.dt.uint16
AF = mybir.ActivationFunctionType
ALU = mybir.AluOpType

N_NODES = 16384
D = 128
N_CORES = 8
K = 10
STRENGTH = 0.1
BLK = 2048
MMW = 512
NEG_DIAG = -8.0


def build_nc(n_nodes=N_NODES, rows=N_NODES // N_CORES, n_cores=N_CORES,
             debug=False):
    nc = bacc.Bacc("TRN2", target_bir_lowering=False, debug=debug,
                   num_devices=n_cores)
    featT = nc.dram_tensor("featT", [D, n_nodes], BF16, kind="ExternalInput")
    feat_rows = nc.dram_tensor("feat_rows", [rows, D], FP32,
                               kind="ExternalInput")
    WT = nc.dram_tensor("WT", [D, D], BF16, kind="ExternalInput")
    out_rows = nc.dram_tensor("out_rows", [rows, D], FP32,
                              kind="ExternalOutput")
    projdram = nc.dram_tensor("projdram", [n_nodes, D], BF16)

    with tile.TileContext(nc) as tc, ExitStack() as ctx:
        _build(ctx, tc, featT.ap(), feat_rows.ap(), WT.ap(),
               out_rows.ap(), projdram.ap(), n_nodes, rows)
    nc.compile()
    return nc


def _build(ctx, tc, featT, feat_rows, WT, out_rows, projdram, n_nodes,
           rows):
    nc = tc.nc
    n_blocks = n_nodes // BLK          # 8
    n_tiles = rows // 128              # 16
    CW = n_blocks * 8                  # candidates per row (64)
    NK = n_tiles * K                   # gathered rows per partition (160)
    GCOLS = 5                          # gather columns per dma_gather

    consts = ctx.enter_context(tc.tile_pool(name="consts", bufs=1))
    bigbuf = ctx.enter_context(tc.tile_pool(name="bigbuf", bufs=1))
    stream = ctx.enter_context(tc.tile_pool(name="stream", bufs=2))
    small = ctx.enter_context(tc.tile_pool(name="small", bufs=3))
    perts = ctx.enter_context(tc.tile_pool(name="perts", bufs=1))
    gpool = ctx.enter_context(tc.tile_pool(name="gpool", bufs=8))
    psum_blk = ctx.enter_context(
        tc.tile_pool(name="psum_blk", bufs=2, space="PSUM"))

    # ---------------- constants ----------------
    ident32 = consts.tile([128, 128], FP32)
    make_identity(nc, ident32[:])
    ident = consts.tile([128, 128], BF16)
    nc.vector.tensor_copy(ident[:], ident32[:])
    negI = consts.tile([128, 128], BF16)
    nc.gpsimd.memset(negI[:], 0.0)
    nc.gpsimd.affine_select(
        out=negI[:], in_=negI[:], compare_op=ALU.not_equal, fill=NEG_DIAG,
        base=0, pattern=[[-1, 128]], channel_multiplier=1)

    maskc = consts.tile([128, 1], I32)      # ~0x7FF
    nc.gpsimd.memset(maskc[:], -2048.0)
    m8c16 = consts.tile([128, 1], I16)      # ~7
    nc.gpsimd.memset(m8c16[:], -8.0)
    sh8c16 = consts.tile([128, 1], I16)     # shift 8
    nc.gpsimd.memset(sh8c16[:], 8.0)
    lowc16 = consts.tile([128, 1], I16)     # 0x7FF
    nc.gpsimd.memset(lowc16[:], 2047.0)

    WT_sb = consts.tile([D, D], BF16)
    nc.sync.dma_start(out=WT_sb[:], in_=WT)

    # ---------------- prep ----------------
    projT = bigbuf.tile([128, n_nodes], FP32)
    pnat = bigbuf.tile([128, n_nodes], BF16)
    nrmT = bigbuf.tile([128, n_nodes], BF16)
    projdram_v = projdram.rearrange("(c p) d -> p c d", p=128)

    for b in range(n_blocks):
        fT = stream.tile([128, BLK], BF16, tag="ftblk")
        nc.sync.dma_start(out=fT[:], in_=featT[:, b * BLK:(b + 1) * BLK])
        pA = psum_blk.tile([128, BLK], FP32, tag="blk")
        for m in range(BLK // MMW):
            nc.tensor.matmul(pA[:, m * MMW:(m + 1) * MMW], lhsT=WT_sb[:],
                             rhs=fT[:, m * MMW:(m + 1) * MMW], start=True,
                             stop=True)
        nc.scalar.activation(projT[:, b * BLK:(b + 1) * BLK], pA[:],
                             AF.Copy, scale=STRENGTH)

    nchunks = n_nodes // 128
    ngrp = nchunks // 4
    ssq_all = perts.tile([128, nchunks], FP32)
    inv_all = perts.tile([128, nchunks], FP32)
    for g in range(ngrp):
        pT = psum_blk.tile([128, BLK], FP32, tag="blk")
        pG = pT[:, 0:512]
        for j in range(4):
            c = 4 * g + j
            nc.tensor.transpose(pG[:, 128 * j:128 * (j + 1)],
                                projT[:, c * 128:(c + 1) * 128], ident32[:])
        nc.scalar.copy(pnat[:, g * 512:(g + 1) * 512], pG)
        nc.sync.dma_start(
            out=projdram_v[:, 4 * g:4 * (g + 1), :],
            in_=pnat[:, g * 512:(g + 1) * 512].rearrange(
                "p (c d) -> p c d", d=128))
        sq = stream.tile([128, 512], FP32, tag="sqg")
        nc.scalar.activation(sq[:], pG, AF.Square)
        nc.vector.tensor_reduce(
            ssq_all[:, 4 * g:4 * (g + 1)],
            sq[:].rearrange("p (c d) -> p c d", d=128),
            axis=mybir.AxisListType.X, op=ALU.add)
    nrm_all = perts.tile([128, nchunks], FP32)
    for g in range(ngrp):
        nc.scalar.sqrt(nrm_all[:, 4 * g:4 * (g + 1)],
                       ssq_all[:, 4 * g:4 * (g + 1)])
        nc.vector.reciprocal(inv_all[:, 4 * g:4 * (g + 1)],
                             nrm_all[:, 4 * g:4 * (g + 1)])
    def phase2_group(g):
        pT = psum_blk.tile([128, BLK], FP32, tag="blk")
        pG2 = pT[:, 1024:1536]
        for j in range(4):
            c = 4 * g + j
            nch = small.tile([128, 128], FP32, tag="nch")
            nc.scalar.activation(nch[:], pnat[:, c * 128:(c + 1) * 128],
                                 AF.Copy, scale=inv_all[:, c:c + 1])
            nc.tensor.transpose(pG2[:, 128 * j:128 * (j + 1)], nch[:],
                                ident32[:])
        nc.scalar.copy(nrmT[:, g * 512:(g + 1) * 512], pG2)


    # ---------------- main ----------------
    E_all = perts.tile([128, NK], FP32)
    invZ01 = perts.tile([128, n_tiles], FP32)
    gidx16 = perts.tile([128, NK], I16)
    idxw = perts.tile([128, NK * 8], I16)
    G_tiles = [None] * n_tiles

    def issue_tile_gather(t):
        c0 = t * K
        w0 = c0 * 8
        wid = K * 8
        for a in range(8):
            nc.sync.dma_start(
                out=idxw[0:16, w0 + a:w0 + wid:8],
                in_=gidx16[16 * a:16 * (a + 1), c0:c0 + K])
        for q in range(1, 8):
            nc.sync.dma_start(out=idxw[16 * q:16 * (q + 1), w0:w0 + wid],
                              in_=idxw[0:16, w0:w0 + wid])
        Gs = []
        for h in range(2):
            col0 = t * K + h * GCOLS
            G = gpool.tile([128, GCOLS, D], BF16, tag="gath")
            nc.gpsimd.dma_gather(
                out_ap=G[:], in_ap=projdram,
                idxs_ap=idxw[:, col0 * 8:(col0 + GCOLS) * 8],
                num_idxs=GCOLS * 128, num_idxs_reg=GCOLS * 128,
                elem_size=D, queue_num=0)
            Gs.append(G)
        G_tiles[t] = Gs

    def issue_apply(t):
        Gs = G_tiles[t]
        acc = small.tile([128, D], FP32, tag="acc")
        nc.vector.scalar_tensor_tensor(
            out=acc[:], in0=Gs[0][:, 0, :].squeeze(),
            scalar=E_all[:, t * K:t * K + 1], in1=acc[:],
            op0=ALU.mult, op1=ALU.bypass)
        for j in range(1, K):
            nc.vector.scalar_tensor_tensor(
                out=acc[:], in0=Gs[j // GCOLS][:, j % GCOLS, :].squeeze(),
                scalar=E_all[:, t * K + j:t * K + j + 1], in1=acc[:],
                op0=ALU.mult, op1=ALU.add)
        ft = small.tile([128, D], FP32, tag="ft")
        nc.sync.dma_start(out=ft[:],
                          in_=feat_rows[t * 128:(t + 1) * 128, :])
        o = small.tile([128, D], FP32, tag="oo")
        nc.vector.scalar_tensor_tensor(
            out=o[:], in0=acc[:], scalar=invZ01[:, t:t + 1], in1=ft[:],
            op0=ALU.mult, op1=ALU.add)
        nc.sync.dma_start(out=out_rows[t * 128:(t + 1) * 128, :], in_=o[:])


    tile_cand = [perts.tile([128, CW], FP32, name=f"candA{i}")
                 for i in range(2)]
    tile_cidx = [perts.tile([128, CW], U16, name=f"cidxA{i}")
                 for i in range(2)]

    def topk_block(t, b):
        lhs = nrmT[:, t * 128:(t + 1) * 128]
        b_diag = (t * 128) // BLK
        off = (t * 128) % BLK
        ps = psum_blk.tile([128, BLK], FP32, tag="blk")
        for m in range(BLK // MMW):
            is_diag_chunk = (b == b_diag and m == off // MMW)
            nc.tensor.matmul(
                ps[:, m * MMW:(m + 1) * MMW], lhsT=lhs,
                rhs=nrmT[:, b * BLK + m * MMW:b * BLK + (m + 1) * MMW],
                start=True, stop=not is_diag_chunk)
            if is_diag_chunk:
                nc.tensor.matmul(
                    ps[:, off:off + 128], lhsT=negI[:], rhs=ident[:],
                    start=False, stop=True, skip_group_check=True)
        cand = tile_cand[t % 2]
        cidx = tile_cidx[t % 2]
        nc.vector.max(out=cand[:, b * 8:(b + 1) * 8], in_=ps[:])
        nc.vector.max_index(out=cidx[:, b * 8:(b + 1) * 8],
                            in_max=cand[:, b * 8:(b + 1) * 8],
                            in_values=ps[:])

    def refine_tile(t):
        cand = tile_cand[t % 2]
        cidx = tile_cidx[t % 2]
        # pack candidates: (value_bits & -2048) | local_idx
        cidx32 = small.tile([128, CW], I32, tag="cidx32")
        nc.vector.tensor_copy(cidx32[:], cidx[:])
        candP = small.tile([128, CW], I32, tag="candP")
        nc.vector.scalar_tensor_tensor(
            out=candP[:], in0=cand[:].bitcast(I32), scalar=maskc[:, 0:1],
            in1=cidx32[:], op0=ALU.bitwise_and, op1=ALU.bitwise_or)
        candPf = candP[:].bitcast(FP32)

        P16 = small.tile([128, 16], FP32, tag="p16")
        pos16 = small.tile([128, 16], U16, tag="pos16")
        nc.vector.max(out=P16[:, 0:8], in_=candPf)
        nc.vector.max_index(out=pos16[:, 0:8], in_max=P16[:, 0:8],
                            in_values=candPf)
        cand2 = small.tile([128, CW], FP32, tag="cand2")
        nc.vector.match_replace(out=cand2[:], in_to_replace=P16[:, 0:8],
                                in_values=candPf, imm_value=-3.0e38)
        nc.vector.max(out=P16[:, 8:16], in_=cand2[:])
        nc.vector.max_index(out=pos16[:, 8:16], in_max=P16[:, 8:16],
                            in_values=cand2[:])

        base = small.tile([128, K], I16, tag="base")
        nc.vector.scalar_tensor_tensor(
            out=base[:], in0=pos16[:, 0:K].bitcast(I16),
            scalar=m8c16[:, 0:1], in1=base[:],
            op0=ALU.bitwise_and, op1=ALU.bypass)
        loc = small.tile([128, K], I16, tag="loc")
        nc.vector.scalar_tensor_tensor(
            out=loc[:], in0=P16[:].bitcast(I16)[:, 0:2 * K:2],
            scalar=lowc16[:, 0:1], in1=loc[:],
            op0=ALU.bitwise_and, op1=ALU.bypass)
        nc.vector.scalar_tensor_tensor(
            out=gidx16[:, t * K:(t + 1) * K], in0=base[:],
            scalar=sh8c16[:, 0:1], in1=loc[:],
            op0=ALU.logical_shift_left, op1=ALU.bitwise_or)

        E10 = E_all[:, t * K:(t + 1) * K]
        Z = small.tile([128, 1], FP32, tag="zz")
        nc.scalar.activation(E10, P16[:, 0:K], AF.Exp, accum_out=Z[:])
        nc.vector.reciprocal(invZ01[:, t:t + 1], Z[:])

    # ---- weave: phase-2 groups with tile-0 topk blocks ----
    # topk(0, b) needs nrmT groups [4b, 4b+4) plus group 0 (lhs).
    phase2_group(0)
    phase2_group(1)
    for g in range(2, ngrp):
        phase2_group(g)
        if g % 4 == 1 and g >= 5:
            topk_block(0, (g - 5) // 4)
    topk_block(0, 7)
    refine_tile(0)

    issue_tile_gather(0)
    for t in range(1, n_tiles):
        for b in range(n_blocks):
            topk_block(t, b)
        refine_tile(t)
        issue_tile_gather(t)
        if t >= 2:
            issue_apply(t - 2)
    issue_apply(n_tiles - 2)
    issue_apply(n_tiles - 1)


_NC_CACHE = {}


def _get_nc(n_nodes, rows, n_cores):
    key = (n_nodes, rows, n_cores)
    if key not in _NC_CACHE:
        _NC_CACHE[key] = build_nc(n_nodes, rows, n_cores)
    return _NC_CACHE[key]


def make_in_maps(feat, W, emb, n_cores=N_CORES):
    n = feat.shape[0]
    rows = n // n_cores
    featT = np.ascontiguousarray(feat.T.astype(ml_dtypes.bfloat16))
    WT = np.ascontiguousarray(W.T.astype(ml_dtypes.bfloat16))
    emb = np.ascontiguousarray(emb.astype(np.float32))
    maps = []
    for c in range(n_cores):
        maps.append({
            "featT": np.ascontiguousarray(np.roll(featT, -rows * c, axis=1)),
            "feat_rows": np.ascontiguousarray(
                feat[rows * c:rows * (c + 1)].astype(np.float32)
                + STRENGTH * emb.astype(np.float32)),
            "WT": WT,
        })
    return maps


def kernel(feat, W, emb):
    feat = np.asarray(feat, dtype=np.float32)
    W = np.asarray(W, dtype=np.float32)
    emb = np.asarray(emb, dtype=np.float32)
    n = feat.shape[0]
    rows = n // N_CORES
    nc = _get_nc(n, rows, N_CORES)
    in_maps = make_in_maps(feat, W, emb, N_CORES)
    res = run_bass_kernel_spmd(nc, in_maps, core_ids=list(range(N_CORES)))
    out = np.concatenate([res.results[c]["out_rows"] for c in range(N_CORES)],
                         axis=0)
    return out.astype(np.float32)

